# revision 1
# baseline (speedup 1.0000x reference)
"""Sparse (sliding-window + sink) GQA attention block on 8 TRN2 NeuronCores.

Sharding: tensor-parallel over the 64 q-heads -> 8 q-heads (= 1 kv-head
group) per core; x replicated; wo partial outputs summed on host.

Per-core dataflow (matmuls in float32r/TF32, storage f32):
  qT[e,s]  = wqT^T x^T   (contraction d on partitions)
  k/vT     = wkvT^T x^T  (k rows 0:64, v rows 64:128 of one [128,1024] tile)
  RoPE on qT/kT (rotate-half swap via SBUF->SBUF DMA, tables from host)
  v natural via PE transpose of vT; ones column appended -> denom for free
  scoresT[j,i] per (h,J): lhsT=kT[:,J], rhs=qT_h[:, J:J+256] (window => 2 blocks)
  eT = exp(scoresT + mask)  (softmax shift-invariance: no max pass)
  attnT/denom = v_aug^T eT into one [65, 2048] psum per head
  window-overlap add, r = 1/(denom+esink) broadcast by ones-matmul, scale,
  out[i,dd] = attnT^T woT -> partial [1024, 2880].  Host: sum partials + wo_b.
"""

import numpy as np

B, S, DIM = 1, 1024, 2880
H, HKV, HD = 64, 8, 64
GROUP = H // HKV
WINDOW = 128
THETA = 150000.0
NC = 8
HL = H // NC                 # 8 local q-heads per core
EL = HL * HD                 # 512 local q-dim
DT = (DIM + 127) // 128      # 23 d-tiles (22 full + 64)
NJ = S // 128                # 8 j/i blocks
MASK_NEG = -1.0e4

_cache = {}


def _build_module():
    import concourse.bacc as bacc
    import concourse.mybir as mybir
    import concourse.tile as tile

    f32 = mybir.dt.float32
    f32r = mybir.dt.float32r
    AF = mybir.ActivationFunctionType
    OP = mybir.AluOpType

    nc = bacc.Bacc("TRN2", target_bir_lowering=False, debug=False)

    def din(name, shape, dt=f32):
        return nc.dram_tensor(name, shape, dt, kind="ExternalInput").ap()

    xT = din("xT", [DIM, S], f32r)           # x^T
    wqT = din("wqT", [128, DT * EL], f32r)   # tiled: [p, 512*t + e]
    wkvT = din("wkvT", [128, DT * 128], f32r)  # tiled: [p, 128*t + (k|v)]
    woT = din("woT", [128, 4 * DIM], f32r)   # tiled: [p, 2880*et + dd]
    qb = din("qb", [128, 4])
    kvb = din("kvb", [128, 1])
    cosq = din("cosq", [128, S])             # 0.125-scaled
    sinq = din("sinq", [128, S])             # 0.125-scaled, sign-baked
    cosk = din("cosk", [64, S])
    sinkt = din("sinkt", [64, S])
    maskT = din("maskT", [128, 256])
    esink = din("esink", [128, 2])           # exp(sinks), row 32*(h%4), col h//4
    id64 = din("id64", [128, 64])            # eye(64) stacked twice
    out_d = nc.dram_tensor("out", [S, DIM], f32, kind="ExternalOutput").ap()

    with tile.TileContext(nc) as tc:
        import contextlib
        with contextlib.ExitStack() as ctx:
            res = ctx.enter_context(tc.tile_pool(name="res", bufs=1))
            wq_sb = res.tile([128, DT * EL], f32r, tag="wq")
            wkv_sb = res.tile([128, DT * 128], f32r, tag="wkv")
            cq_sb = res.tile([128, S], f32, tag="cq")
            sq_sb = res.tile([128, S], f32, tag="sq")
            ck_sb = res.tile([64, S], f32, tag="ck")
            sk_sb = res.tile([64, S], f32, tag="sk")
            mk_sb = res.tile([128, 256], f32, tag="mk")
            qb_sb = res.tile([128, 4], f32, tag="qb")
            kvb_sb = res.tile([128, 1], f32, tag="kvb")
            es_sb = res.tile([128, 2], f32, tag="es")
            id_sb = res.tile([128, 64], f32, tag="id")
            ones0 = res.tile([128, 128], f32, tag="ones0")
            ones_sb = res.tile([128, 128], f32, tag="ones")
            qT = [res.tile([128, S], f32, tag=f"qT{i}", name=f"qT{i}")
                  for i in range(4)]
            kv_sb = res.tile([128, S], f32, tag="kv")
            kv2_sb = res.tile([128, S], f32, tag="kv2")
            v_sb = [res.tile([128, 65], f32, tag=f"v{j}", name=f"v{j}")
                    for j in range(NJ)]
            at_pair = [res.tile([128, S], f32, tag=f"at{t}", name=f"at{t}")
                       for t in range(4)]
            dn_ab = [res.tile([128, S], f32, tag=f"dn{i}", name=f"dn{i}")
                     for i in range(2)]
            rinv_ab = [res.tile([128, S], f32, tag=f"ri{i}", name=f"ri{i}")
                       for i in range(2)]
            wo_sb = res.tile([128, 4 * DIM], f32r, tag="wo")

            nc.sync.dma_start(wq_sb[:], wqT[:])
            nc.sync.dma_start(wkv_sb[:], wkvT[:])
            nc.sync.dma_start(cq_sb[:], cosq[:])
            nc.sync.dma_start(sq_sb[:], sinq[:])
            nc.sync.dma_start(ck_sb[:], cosk[:])
            nc.sync.dma_start(sk_sb[:], sinkt[:])
            nc.sync.dma_start(mk_sb[:], maskT[:])
            nc.sync.dma_start(qb_sb[:], qb[:])
            nc.sync.dma_start(kvb_sb[:], kvb[:])
            nc.sync.dma_start(es_sb[:], esink[:])
            nc.sync.dma_start(id_sb[:], id64[:])
            nc.vector.memset(ones0[:], 1.0)
            nc.vector.tensor_copy(ones_sb[:].bitcast(f32r), ones0[:])
            nc.vector.memset(dn_ab[0][:], 1.0)
            nc.vector.memset(dn_ab[1][:], 1.0)

            # ---------------- Phase A: projections ----------------
            with tc.tile_pool(name="xh", bufs=3) as xh_pool, \
                 tc.tile_pool(name="pqA", bufs=1, space="PSUM") as pq_pool, \
                 tc.tile_pool(name="pkvA", bufs=1, space="PSUM") as pkv_pool:
                for sc in range(2):
                    pq = [pq_pool.tile([128, 512], f32, tag=f"pq{e}",
                                       name=f"pq{e}") for e in range(4)]
                    pkv = pkv_pool.tile([128, 512], f32, tag="pkv")
                    for t in range(DT):
                        dp = 128 if t < DT - 1 else DIM - 128 * (DT - 1)
                        xh = xh_pool.tile([128, 512], f32r, tag="xh")
                        nc.sync.dma_start(
                            xh[:dp, :], xT[128 * t:128 * t + dp,
                                           512 * sc:512 * (sc + 1)])
                        rhs = xh[:dp, :]
                        st, sp = (t == 0), (t == DT - 1)
                        for et in range(4):
                            nc.tensor.matmul(
                                pq[et][:],
                                wq_sb[:dp, EL * t + 128 * et:
                                      EL * t + 128 * (et + 1)],
                                rhs, start=st, stop=sp)
                        nc.tensor.matmul(
                            pkv[:], wkv_sb[:dp, 128 * t:128 * (t + 1)],
                            rhs, start=st, stop=sp)
                    for et in range(4):
                        nc.vector.tensor_scalar_add(
                            qT[et][:, 512 * sc:512 * (sc + 1)].bitcast(f32r),
                            pq[et][:], qb_sb[:, et:et + 1])
                    nc.vector.tensor_scalar_add(
                        kv_sb[:, 512 * sc:512 * (sc + 1)].bitcast(f32r),
                        pkv[:], kvb_sb[:, 0:1])

            # ---------------- Phase B: RoPE (swap via SBUF->SBUF DMA) -----
            with tc.tile_pool(name="rope", bufs=1) as rp:
                for et in range(4):
                    q = qT[et]
                    qsw = rp.tile([128, S], f32, tag="qsw")
                    nc.sync.dma_start(qsw[0:32, :], q[32:64, :])
                    nc.sync.dma_start(qsw[32:64, :], q[0:32, :])
                    nc.sync.dma_start(qsw[64:96, :], q[96:128, :])
                    nc.sync.dma_start(qsw[96:128, :], q[64:96, :])
                    tmp = rp.tile([128, S], f32, tag="tmp")
                    qc = rp.tile([128, S], f32, tag="qc")
                    nc.vector.tensor_tensor(tmp[:], qsw[:], sq_sb[:],
                                            op=OP.mult)
                    nc.vector.tensor_tensor(qc[:], q[:], cq_sb[:], op=OP.mult)
                    nc.vector.tensor_tensor(q[:].bitcast(f32r), qc[:], tmp[:],
                                            op=OP.add)
                # k rope (rows 0:64 of kv_sb)
                ksw = rp.tile([64, S], f32, tag="ksw")
                nc.sync.dma_start(ksw[0:32, :], kv_sb[32:64, :])
                nc.sync.dma_start(ksw[32:64, :], kv_sb[0:32, :])
                tmp = rp.tile([128, S], f32, tag="tmp")
                qc = rp.tile([128, S], f32, tag="qc")
                nc.vector.tensor_tensor(tmp[0:64], ksw[:], sk_sb[:],
                                        op=OP.mult)
                nc.vector.tensor_tensor(qc[0:64], kv_sb[0:64], ck_sb[:],
                                        op=OP.mult)
                nc.vector.tensor_tensor(kv_sb[0:64].bitcast(f32r), qc[0:64],
                                        tmp[0:64], op=OP.add)
                # kT copy at base 64 for odd heads (bit-copy of rounded data)
                nc.sync.dma_start(kv2_sb[64:128, :].bitcast(f32r),
                                  kv_sb[0:64, :].bitcast(f32r))

            # ---------------- Phase B2: v transposes ----------------
            with tc.tile_pool(name="pvt", bufs=2, space="PSUM") as pvt_pool:
                for j in range(NJ):
                    pvt = pvt_pool.tile([128, 64], f32, tag="pvt")
                    nc.tensor.transpose(
                        pvt[:], kv_sb[64:128, 128 * j:128 * (j + 1)],
                        id_sb[64:128, :])
                    nc.vector.tensor_copy(v_sb[j][:, 0:64].bitcast(f32r),
                                          pvt[:])
                    nc.vector.tensor_copy(v_sb[j][:, 64:65].bitcast(f32r),
                                          ones0[:, 0:1])

            # ---------------- Phase C: attention ----------------
            with tc.tile_pool(name="pbig", bufs=1, space="PSUM") as pbig_pool, \
                 tc.tile_pool(name="psc", bufs=2, space="PSUM") as ps_pool, \
                 tc.tile_pool(name="att", bufs=2) as att_pool, \
                 tc.tile_pool(name="set", bufs=3) as set_pool:
                for h in range(HL):
                    qt = qT[h // 2]
                    r0 = 64 * (h % 2)
                    tpi = h // 2
                    kt = kv_sb if h % 2 == 0 else kv2_sb
                    pbig = pbig_pool.tile([65, 2048], f32, tag="pbig")
                    for J in range(NJ):
                        ni = 256 if J < NJ - 1 else 128
                        ps = ps_pool.tile([128, 512], f32, tag="ps")
                        nc.tensor.matmul(
                            ps[:, :ni],
                            kt[r0:r0 + 64, 128 * J:128 * (J + 1)]
                            .bitcast(f32r),
                            qt[r0:r0 + 64, 128 * J:128 * J + ni]
                            .bitcast(f32r),
                            start=True, stop=True)
                        sT = set_pool.tile([128, 256], f32, tag="sT")
                        nc.vector.tensor_tensor(sT[:, :ni], ps[:, :ni],
                                                mk_sb[:, :ni], op=OP.add)
                        eT = set_pool.tile([128, 256], f32, tag="eT")
                        nc.scalar.activation(eT[:, :ni].bitcast(f32r),
                                             sT[:, :ni], AF.Exp)
                        nc.tensor.matmul(
                            pbig[:, 256 * J:256 * J + ni],
                            v_sb[J][:, 0:65].bitcast(f32r),
                            eT[:, :ni].bitcast(f32r),
                            start=True, stop=True)
                    # ---- epilogue: window-overlap adds straight off PSUM
                    dst = at_pair[tpi]
                    dnt = dn_ab[h // 4]
                    dr = 32 * (h % 4)
                    esap = es_sb[dr:dr + 1, (h // 4):(h // 4) + 1]
                    aa_r = att_pool.tile([128, 896], f32, tag="aa")
                    pb_a = pbig[0:64, :].rearrange("p (J x) -> p J x", x=256)
                    pb_d = pbig[64:65, :].rearrange("p (J x) -> p J x", x=256)
                    # right halves of J=0..6 -> SBUF (ACT), then add to left
                    nc.scalar.activation(
                        aa_r[r0:r0 + 64, :].rearrange("p (a b) -> p a b",
                                                      b=128),
                        pb_a[:, 0:7, 128:256], AF.Copy)
                    nc.vector.tensor_copy(
                        dst[r0:r0 + 64, 0:128].bitcast(f32r),
                        pbig[0:64, 0:128])
                    nc.vector.tensor_tensor(
                        dst[r0:r0 + 64, 128:1024].bitcast(f32r)
                        .rearrange("p (a b) -> p a b", b=128),
                        pb_a[:, 1:8, 0:128],
                        aa_r[r0:r0 + 64, :].rearrange("p (a b) -> p a b",
                                                      b=128),
                        op=OP.add)
                    # denom row
                    dnr = att_pool.tile([128, 896], f32, tag="dnr")
                    nc.scalar.activation(
                        dnr[dr:dr + 1, :].rearrange("p (a b) -> p a b",
                                                    b=128),
                        pb_d[:, 0:7, 128:256], AF.Copy)
                    nc.vector.tensor_scalar_add(dnt[dr:dr + 1, 0:128],
                                                pbig[64:65, 0:128], esap)
                    nc.vector.scalar_tensor_tensor(
                        dnt[dr:dr + 1, 128:1024].rearrange(
                            "p (a b) -> p a b", b=128),
                        pb_d[:, 1:8, 0:128], esap,
                        dnr[dr:dr + 1, :].rearrange("p (a b) -> p a b",
                                                    b=128),
                        op0=OP.add, op1=OP.add)

                with nc.allow_low_precision(reason="f32r output for matmul"):
                    nc.vector.reciprocal(rinv_ab[0][:].bitcast(f32r),
                                         dn_ab[0][:])
                    nc.vector.reciprocal(rinv_ab[1][:].bitcast(f32r),
                                         dn_ab[1][:])
                # broadcast r (ones-matmul at base 0) and scale attnT
                with tc.tile_pool(name="rst", bufs=2) as rst_pool:
                    for h in range(HL):
                        t, r0 = h // 2, 64 * (h % 2)
                        dr = 32 * (h % 4)
                        stg = rst_pool.tile([1, S], f32, tag="stg",
                                            name=f"stg{h}")
                        nc.sync.dma_start(
                            stg[:].bitcast(f32r),
                            rinv_ab[h // 4][dr:dr + 1, :].bitcast(f32r))
                        for half in range(2):
                            prt = ps_pool.tile([128, 512], f32, tag="ps")
                            nc.tensor.matmul(
                                prt[0:64, :],
                                ones_sb[0:1, 0:64].bitcast(f32r),
                                stg[0:1, 512 * half:512 * (half + 1)]
                                .bitcast(f32r),
                                start=True, stop=True)
                            nc.vector.tensor_tensor(
                                at_pair[t][r0:r0 + 64,
                                           512 * half:512 * (half + 1)]
                                .bitcast(f32r),
                                at_pair[t][r0:r0 + 64,
                                           512 * half:512 * (half + 1)],
                                prt[0:64, :], op=OP.mult)

            # ---------------- Phase D: output projection ----------------
            nc.sync.dma_start(wo_sb[:], woT[:])
            NDD = 6
            DDC = DIM // NDD  # 480
            with tc.tile_pool(name="po", bufs=3, space="PSUM") as po_pool, \
                 tc.tile_pool(name="ob", bufs=3) as ob_pool:
                for it in range(NJ):
                    for dd in range(NDD):
                        po = po_pool.tile([128, DDC], f32, tag="po")
                        for et in range(4):
                            nc.tensor.matmul(
                                po[:],
                                at_pair[et][:, 128 * it:128 * (it + 1)]
                                .bitcast(f32r),
                                wo_sb[:, DIM * et + DDC * dd:
                                      DIM * et + DDC * (dd + 1)],
                                start=(et == 0), stop=(et == 3))
                        ob = ob_pool.tile([128, DDC], f32, tag="ob")
                        nc.scalar.activation(ob[:], po[:], AF.Copy)
                        nc.sync.dma_start(
                            out_d[128 * it:128 * (it + 1),
                                  DDC * dd:DDC * (dd + 1)], ob[:])

    nc.compile()
    return nc


def _esink_layout(s8):
    out = np.zeros((128, 2), np.float32)
    for h in range(HL):
        out[32 * (h % 4), h // 4] = np.exp(np.float64(s8[h]))
    return out


def _host_prep(x, wq_w, wq_b, wk_w, wk_b, wv_w, wv_b, wo_w, wo_b, sinks):
    """Build per-core input maps (host-side sharding + layout prep)."""
    f = np.float32
    xT = np.ascontiguousarray(x.reshape(S, DIM).T).astype(f)       # [2880,1024]

    half = HD // 2
    inv_freq = 1.0 / (THETA ** (np.arange(half, dtype=np.float64) * 2.0 / HD))
    ang = np.arange(S, dtype=np.float64)[:, None] * inv_freq       # [S, 32]
    cos_t = np.cos(ang).T.astype(f)                                # [32, S]
    sin_t = np.sin(ang).T.astype(f)
    cos64 = np.concatenate([cos_t, cos_t], 0)                      # [64, S]
    sin64 = np.concatenate([-sin_t, sin_t], 0)
    scale = np.float32(HD ** -0.5)
    cosq = np.concatenate([cos64, cos64], 0) * scale               # [128, S]
    sinq = np.concatenate([sin64, sin64], 0) * scale
    cosk = cos64
    sinkt = sin64

    jj = np.arange(128)[:, None]
    ii = np.arange(256)[None, :]
    allow_l = (jj <= ii) & (ii < 128)
    allow_r = (ii >= 128) & (jj > ii - 128)
    maskT = np.where(allow_l | allow_r, 0.0, MASK_NEG).astype(f)

    id64 = np.tile(np.eye(64, dtype=f), (2, 1))

    def tile_T(w):  # [E, DIM] -> tiled transposed [128, DT*E]
        E = w.shape[0]
        out = np.zeros((128, DT * E), f)
        for t in range(DT):
            dp = min(128, DIM - 128 * t)
            out[:dp, E * t:E * (t + 1)] = w[:, 128 * t:128 * t + dp].T
        return out

    in_maps = []
    for c in range(NC):
        wq_c = wq_w[EL * c:EL * (c + 1)]                  # [512, 2880]
        wkv_c = np.concatenate([wk_w[HD * c:HD * (c + 1)],
                                wv_w[HD * c:HD * (c + 1)]], 0)  # [128, 2880]
        wo_c = np.ascontiguousarray(wo_w[:, EL * c:EL * (c + 1)].T)  # [512,2880]
        woT_t = np.zeros((128, 4 * DIM), f)
        for et in range(4):
            woT_t[:, DIM * et:DIM * (et + 1)] = wo_c[128 * et:128 * (et + 1)]
        in_maps.append({
            "xT": xT,
            "wqT": tile_T(wq_c).astype(f),
            "wkvT": tile_T(wkv_c).astype(f),
            "woT": woT_t,
            "qb": np.ascontiguousarray(
                wq_b[EL * c:EL * (c + 1)].reshape(4, 128).T).astype(f),
            "kvb": np.ascontiguousarray(np.concatenate(
                [wk_b[HD * c:HD * (c + 1)],
                 wv_b[HD * c:HD * (c + 1)]]).reshape(1, 128).T).astype(f),
            "cosq": cosq, "sinq": sinq, "cosk": cosk, "sinkt": sinkt,
            "maskT": maskT,
            "esink": _esink_layout(sinks[HL * c:HL * (c + 1)]),
            "id64": id64,
        })
    return in_maps


def run_on_hw(inputs, trace=False, **kw):
    from concourse import bass_utils
    if "nc" not in _cache:
        _cache["nc"] = _build_module()
    in_maps = _host_prep(**inputs)
    res = bass_utils.run_bass_kernel_spmd(
        _cache["nc"], in_maps, core_ids=list(range(NC)), trace=trace, **kw)
    partials = [res.results[c]["out"] for c in range(NC)]
    out = np.sum(np.stack(partials, 0), 0, dtype=np.float64)
    out = (out + inputs["wo_b"].astype(np.float64)).astype(np.float32)
    return out.reshape(B, S, DIM), res


def kernel(**inputs) -> np.ndarray:
    out, _ = run_on_hw(inputs, trace=False)
    return out



# revision 27
# speedup vs baseline: 1.0861x; 1.0861x over previous
"""Sparse (sliding-window + sink) GQA attention block on 8 TRN2 NeuronCores.

Sharding: tensor-parallel over the 64 q-heads -> 8 q-heads (= 1 kv-head
group) per core; x replicated; wo partial outputs summed on host.

v2 dataflow (bf16 storage for DMA-heavy operands, f32 rope/scores path):
  Phase A: qT/k/vT = W^T x^T streamed in bf16 chunks (weights chunk-DMA'd
    and interleaved with x so the PE starts ~4us in); bias adds on GpSimd.
  RoPE per 512-col half on DVE, overlapped with phase A's second half.
  Attention per head: scoresT (f32r) -> +mask (GpSimd) -> exp (ACT, bf16)
    -> window accumulated in PSUM with two 128-col matmuls per i-block
    (no SBUF overlap-adds); denom row via ones-column of v_aug.
  Epilogue: denom rows packed [8,S]; reciprocal_approx_fast; rinv
    broadcast by ones-matmul; scale into bf16 atb tiles.
  Phase D: out[i,dd] partial = atb^T wo (bf16), written bf16; host sums.
"""

import numpy as np

B, S, DIM = 1, 1024, 2880
H, HKV, HD = 64, 8, 64
GROUP = H // HKV
WINDOW = 128
THETA = 150000.0
NC = 8
HL = H // NC                 # 8 local q-heads per core
EL = HL * HD                 # 512 local q-dim
DT = (DIM + 127) // 128      # 23 d-tiles (22 full + 64)
NJ = S // 128                # 8 j/i blocks
MASK_NEG = -1.0e4

_cache = {}


def _build_module():
    import concourse.bacc as bacc
    import concourse.mybir as mybir
    import concourse.tile as tile

    f32 = mybir.dt.float32
    f32r = mybir.dt.float32r
    bf16 = mybir.dt.bfloat16
    AF = mybir.ActivationFunctionType
    OP = mybir.AluOpType

    nc = bacc.Bacc("TRN2", target_bir_lowering=False, debug=False)

    def din(name, shape, dt=f32):
        return nc.dram_tensor(name, shape, dt, kind="ExternalInput").ap()

    xT = din("xT", [DIM, S], bf16)             # x^T
    wqT = din("wqT", [128, DT * EL], bf16)     # tiled: [p, 512*t + e]
    wkvT = din("wkvT", [128, DT * 128], bf16)  # tiled: [p, 128*t + (k|v)]
    woT = din("woT", [128, 4 * DIM], bf16)     # tiled: [p, 2880*et + dd]
    qb = din("qb", [128, 4])
    kvb = din("kvb", [128, 1])
    cosq = din("cosq", [128, S])               # 0.125-scaled
    sinq = din("sinq", [128, S])               # 0.125-scaled, sign-baked
    cosk = din("cosk", [64, S])
    sinkt = din("sinkt", [64, S])
    maskT = din("maskT", [128, 1024])          # [std|std | std|J7]
    esink = din("esink", [128, 2])             # exp(sinks), row 32*(h%4), col h//4
    id64 = din("id64", [128, 64])              # eye(64) stacked twice
    out_d = nc.dram_tensor("out", [S, DIM], bf16, kind="ExternalOutput").ap()

    with tile.TileContext(nc) as tc:
        import contextlib
        with contextlib.ExitStack() as ctx:
            res = ctx.enter_context(tc.tile_pool(name="res", bufs=1))
            wq_sb = res.tile([128, DT * EL], bf16, tag="wq")
            wkv_sb = res.tile([128, DT * 128], bf16, tag="wkv")
            wo_sb = res.tile([128, 4 * DIM], bf16, tag="wo")
            cq_sb = res.tile([128, S], f32, tag="cq")
            sq_sb = res.tile([128, S], f32, tag="sq")
            ck_sb = res.tile([64, S], f32, tag="ck")
            sk_sb = res.tile([64, S], f32, tag="sk")
            mk_sb = res.tile([128, 1024], f32, tag="mk")
            qb_sb = res.tile([128, 4], f32, tag="qb")
            kvb_sb = res.tile([128, 1], f32, tag="kvb")
            es_sb = res.tile([128, 2], f32, tag="es")
            id_sb = res.tile([128, 64], f32, tag="id")
            ones0 = res.tile([1, 64], f32, tag="ones0")
            onesr = res.tile([1, 64], f32, tag="onesr")
            qT = [res.tile([128, S + 128], f32, tag=f"qT{i}", name=f"qT{i}")
                  for i in range(4)]
            kv_sb = res.tile([128, S], f32, tag="kv")
            kv2_sb = res.tile([128, S], f32, tag="kv2")
            v_sb = [res.tile([128, 65], bf16, tag=f"v{j}", name=f"v{j}")
                    for j in range(NJ)]
            at_pair = [res.tile([128, S], f32, tag=f"at{t}", name=f"at{t}")
                       for t in range(4)]
            atb = [res.tile([128, S], bf16, tag=f"ab{t}", name=f"ab{t}")
                   for t in range(4)]
            dn_ab = [res.tile([128, S], f32, tag=f"dn{i}", name=f"dn{i}")
                     for i in range(2)]
            ri_ab = [res.tile([128, S], f32, tag=f"ri{i}", name=f"ri{i}")
                     for i in range(2)]

            nc.vector.memset(ones0[:], 1.0)
            nc.vector.tensor_copy(onesr[:].bitcast(f32r), ones0[:])
            nc.vector.memset(dn_ab[0][:], 1.0)
            nc.vector.memset(dn_ab[1][:], 1.0)
            z0 = res.tile([128, 128], f32, tag="z0")
            nc.vector.memset(z0[:], 0.0)
            for et in range(4):
                nc.vector.tensor_copy(qT[et][:, S:S + 128].bitcast(f32r),
                                      z0[:])

            # ---- initial DMAs: t=0 split fine so the PE starts ~4us in
            for et in range(4):
                nc.sync.dma_start(wq_sb[:, 128 * et:128 * (et + 1)],
                                  wqT[:, 128 * et:128 * (et + 1)])
            nc.sync.dma_start(wkv_sb[:, 0:128], wkvT[:, 0:128])
            nc.sync.dma_start(qb_sb[:], qb[:])
            nc.sync.dma_start(kvb_sb[:], kvb[:])

            def wq_chunk(t):
                nc.sync.dma_start(wq_sb[:, EL * t:EL * (t + 1)],
                                  wqT[:, EL * t:EL * (t + 1)])
                nc.sync.dma_start(wkv_sb[:, 128 * t:128 * (t + 1)],
                                  wkvT[:, 128 * t:128 * (t + 1)])

            for t in range(1, 8):
                wq_chunk(t)

            # ---------------- Phase A + RoPE, per 512-col half -----------
            with tc.tile_pool(name="xh", bufs=4) as xh_pool, \
                 tc.tile_pool(name="pqA", bufs=1, space="PSUM") as pq_pool, \
                 tc.tile_pool(name="pkvA", bufs=1, space="PSUM") as pkv_pool, \
                 tc.tile_pool(name="pvt", bufs=2, space="PSUM") as pvt_pool, \
                 tc.tile_pool(name="rope", bufs=2) as rp:
              for sc in range(2):
                pq = [pq_pool.tile([128, 512], f32, tag=f"pq{e}",
                                   name=f"pq{e}") for e in range(4)]
                pkv = pkv_pool.tile([128, 512], f32, tag="pkv")
                for t in range(DT):
                    dp = 128 if t < DT - 1 else DIM - 128 * (DT - 1)
                    # stream later weight chunks behind the compute
                    if sc == 0 and t < 8:
                        if t + 8 <= DT - 1:
                            wq_chunk(t + 8)
                        if t + 16 <= DT - 1:
                            wq_chunk(t + 16)
                    if sc == 0 and t == 8:
                        nc.sync.dma_start(cq_sb[:], cosq[:])
                        nc.sync.dma_start(sq_sb[:], sinq[:])
                        nc.sync.dma_start(ck_sb[:], cosk[:])
                        nc.sync.dma_start(sk_sb[:], sinkt[:])
                        nc.sync.dma_start(mk_sb[:], maskT[:])
                        nc.sync.dma_start(es_sb[:], esink[:])
                        nc.sync.dma_start(id_sb[:], id64[:])
                    if sc == 0 and 10 <= t < 22:
                        # prefetch wo (2.95MB) in 12 chunks
                        c = t - 10
                        nc.sync.dma_start(wo_sb[:, 960 * c:960 * (c + 1)],
                                          woT[:, 960 * c:960 * (c + 1)])
                    xh = xh_pool.tile([128, 512], bf16, tag="xh")
                    nc.sync.dma_start(
                        xh[:dp, :], xT[128 * t:128 * t + dp,
                                       512 * sc:512 * (sc + 1)])
                    rhs = xh[:dp, :]
                    st, sp = (t == 0), (t == DT - 1)
                    for et in range(4):
                        nc.tensor.matmul(
                            pq[et][:],
                            wq_sb[:dp, EL * t + 128 * et:
                                  EL * t + 128 * (et + 1)],
                            rhs, start=st, stop=sp)
                    nc.tensor.matmul(
                        pkv[:], wkv_sb[:dp, 128 * t:128 * (t + 1)],
                        rhs, start=st, stop=sp)
                hs = slice(512 * sc, 512 * (sc + 1))
                for et in range(4):
                    nc.scalar.activation(qT[et][:, hs].bitcast(f32r),
                                         pq[et][:],
                                         AF.Identity, bias=qb_sb[:, et:et + 1])
                nc.scalar.activation(kv_sb[:, hs].bitcast(f32r), pkv[:],
                                     AF.Identity, bias=kvb_sb[:, 0:1])

                # ---- RoPE for this half (DVE + swap DMAs), k first
                ksw = rp.tile([64, 512], f32, tag="ksw")
                nc.sync.dma_start(ksw[0:32, :], kv_sb[32:64, hs])
                nc.sync.dma_start(ksw[32:64, :], kv_sb[0:32, hs])
                ktmp = rp.tile([64, 512], f32, tag="ktmp")
                kqc = rp.tile([64, 512], f32, tag="kqc")
                nc.vector.tensor_tensor(ktmp[:], ksw[:], sk_sb[:, hs],
                                        op=OP.mult)
                nc.vector.tensor_tensor(kqc[:], kv_sb[0:64, hs],
                                        ck_sb[:, hs], op=OP.mult)
                nc.vector.tensor_tensor(kv_sb[0:64, hs].bitcast(f32r),
                                        kqc[:], ktmp[:], op=OP.add)
                # kT copy at base 64 for odd heads
                nc.sync.dma_start(kv2_sb[64:128, hs].bitcast(f32r),
                                  kv_sb[0:64, hs].bitcast(f32r))
                # v transposes for this half (PE, tiny) -> bf16 v_sb
                for j in range(4 * sc, 4 * sc + 4):
                    pvt = pvt_pool.tile([128, 64], f32, tag="pvt")
                    nc.tensor.transpose(
                        pvt[:], kv_sb[64:128, 128 * j:128 * (j + 1)],
                        id_sb[64:128, :])
                    nc.vector.tensor_copy(v_sb[j][:, 0:64], pvt[:])
                    nc.vector.memset(v_sb[j][:, 64:65], 1.0)
                # q rope
                for et in range(4):
                    q = qT[et]
                    qsw = rp.tile([128, 512], f32, tag="qsw")
                    nc.sync.dma_start(qsw[0:32, :], q[32:64, hs])
                    nc.sync.dma_start(qsw[32:64, :], q[0:32, hs])
                    nc.sync.dma_start(qsw[64:96, :], q[96:128, hs])
                    nc.sync.dma_start(qsw[96:128, :], q[64:96, hs])
                    tmp = rp.tile([128, 512], f32, tag="tmp")
                    qc = rp.tile([128, 512], f32, tag="qc")
                    nc.vector.tensor_tensor(tmp[:], qsw[:], sq_sb[:, hs],
                                            op=OP.mult)
                    nc.vector.tensor_tensor(qc[:], q[:, hs], cq_sb[:, hs],
                                            op=OP.mult)
                    nc.vector.tensor_tensor(q[:, hs].bitcast(f32r),
                                            qc[:], tmp[:], op=OP.add)

            # ---------------- Phase C: attention ----------------
            with tc.tile_pool(name="pbig", bufs=2, space="PSUM") as pbig_pool, \
                 tc.tile_pool(name="psc", bufs=2, space="PSUM") as ps_pool, \
                 tc.tile_pool(name="prt", bufs=2, space="PSUM") as prt_pool, \
                 tc.tile_pool(name="set", bufs=2) as sT_pool, \
                 tc.tile_pool(name="eet", bufs=3) as eT_pool, \
                 tc.tile_pool(name="stg", bufs=2) as stg_pool:

                def epilogue(g):
                    # rinv for head group g (4 heads), then scale attnT
                    nc.vector.reciprocal_approx_fast(ri_ab[g][:], dn_ab[g][:])
                    for h in range(4 * g, 4 * g + 4):
                        t, r0 = h // 2, 64 * (h % 2)
                        dr = 32 * (h % 4)
                        stg = stg_pool.tile([1, S], f32, tag="stg",
                                            name=f"stg{h}")
                        nc.sync.dma_start(stg[:].bitcast(f32r),
                                          ri_ab[g][dr:dr + 1, :]
                                          .bitcast(f32r))
                        for half in range(2):
                            hs = slice(512 * half, 512 * (half + 1))
                            prt = prt_pool.tile([64, 512], f32, tag="prt")
                            nc.tensor.matmul(
                                prt[:], onesr[0:1, :].bitcast(f32r),
                                stg[0:1, hs].bitcast(f32r),
                                start=True, stop=True)
                            nc.vector.tensor_tensor(
                                atb[t][r0:r0 + 64, hs],
                                at_pair[t][r0:r0 + 64, hs],
                                prt[0:64, :], op=OP.mult)

                for h in range(HL):
                    qt = qT[h // 2]
                    r0 = 64 * (h % 2)
                    tpi = h // 2
                    kt = kv_sb if h % 2 == 0 else kv2_sb
                    pbig = pbig_pool.tile([65, S], f32, tag="pbig")
                    eS = [None] * NJ     # AP slices into pair tiles
                    for Jp in range(NJ // 2):
                        # scores for J-pair (2Jp, 2Jp+1) into one psum tile
                        ps = ps_pool.tile([128, 512], f32, tag="ps")
                        for q2 in range(2):
                            J = 2 * Jp + q2
                            nc.tensor.matmul(
                                ps[:, 256 * q2:256 * (q2 + 1)],
                                kt[r0:r0 + 64, 128 * J:128 * (J + 1)]
                                .bitcast(f32r),
                                qt[r0:r0 + 64, 128 * J:128 * J + 256]
                                .bitcast(f32r),
                                start=True, stop=True)
                        mks = (slice(0, 512) if Jp < NJ // 2 - 1
                               else slice(512, 1024))
                        sT = sT_pool.tile([128, 512], f32, tag="sT")
                        nc.vector.tensor_tensor(sT[:], ps[:], mk_sb[:, mks],
                                                op=OP.add)
                        eT = eT_pool.tile([128, 512], bf16, tag="eT")
                        nc.scalar.activation(eT[:], sT[:], AF.Exp)
                        eS[2 * Jp] = eT[:, 0:256]
                        eS[2 * Jp + 1] = eT[:, 256:512]
                        # window-accumulated attnT for i-blocks I=2Jp, 2Jp+1
                        for q2 in range(2):
                            J = 2 * Jp + q2
                            dst = pbig[:, 128 * J:128 * (J + 1)]
                            if J == 0:
                                nc.tensor.matmul(dst, v_sb[0][:, 0:65],
                                                 eS[0][:, 0:128],
                                                 start=True, stop=True)
                            else:
                                nc.tensor.matmul(dst, v_sb[J - 1][:, 0:65],
                                                 eS[J - 1][:, 128:256],
                                                 start=True, stop=False)
                                nc.tensor.matmul(dst, v_sb[J][:, 0:65],
                                                 eS[J][:, 0:128],
                                                 start=False, stop=True)
                    # drain: attnT rows -> at_pair (ACT), denom row -> dn8
                    nc.scalar.activation(at_pair[tpi][r0:r0 + 64, :],
                                         pbig[0:64, :], AF.Copy)
                    dr = 32 * (h % 4)
                    nc.scalar.activation(dn_ab[h // 4][dr:dr + 1, :],
                                         pbig[64:65, :], AF.Identity,
                                         bias=es_sb[dr:dr + 1,
                                                    (h // 4):(h // 4) + 1])
                    if h == 3:
                        epilogue(0)
                epilogue(1)

            # ---------------- Phase D: output projection ----------------
            NDD = 6
            DDC = DIM // NDD  # 480
            with tc.tile_pool(name="po", bufs=3, space="PSUM") as po_pool, \
                 tc.tile_pool(name="ob", bufs=3) as ob_pool:
                for it in range(NJ):
                    for dd in range(NDD):
                        po = po_pool.tile([128, DDC], f32, tag="po")
                        for et in range(4):
                            nc.tensor.matmul(
                                po[:],
                                atb[et][:, 128 * it:128 * (it + 1)],
                                wo_sb[:, DIM * et + DDC * dd:
                                      DIM * et + DDC * (dd + 1)],
                                start=(et == 0), stop=(et == 3))
                        ob = ob_pool.tile([128, DDC], bf16, tag="ob")
                        nc.scalar.activation(ob[:], po[:], AF.Copy)
                        nc.sync.dma_start(
                            out_d[128 * it:128 * (it + 1),
                                  DDC * dd:DDC * (dd + 1)], ob[:])

    nc.compile()
    return nc


def _esink_layout(s8):
    out = np.zeros((128, 2), np.float32)
    for h in range(HL):
        out[32 * (h % 4), h // 4] = np.exp(np.float64(s8[h]))
    return out


def _host_prep(x, wq_w, wq_b, wk_w, wk_b, wv_w, wv_b, wo_w, wo_b, sinks):
    """Build per-core input maps (host-side sharding + layout prep)."""
    import ml_dtypes
    f = np.float32
    bf = ml_dtypes.bfloat16
    xT = np.ascontiguousarray(x.reshape(S, DIM).T).astype(bf)      # [2880,1024]

    half = HD // 2
    inv_freq = 1.0 / (THETA ** (np.arange(half, dtype=np.float64) * 2.0 / HD))
    ang = np.arange(S, dtype=np.float64)[:, None] * inv_freq       # [S, 32]
    cos_t = np.cos(ang).T.astype(f)                                # [32, S]
    sin_t = np.sin(ang).T.astype(f)
    cos64 = np.concatenate([cos_t, cos_t], 0)                      # [64, S]
    sin64 = np.concatenate([-sin_t, sin_t], 0)
    scale = np.float32(HD ** -0.5)
    cosq = np.concatenate([cos64, cos64], 0) * scale               # [128, S]
    sinq = np.concatenate([sin64, sin64], 0) * scale
    cosk = cos64
    sinkt = sin64

    jj = np.arange(128)[:, None]
    ii = np.arange(256)[None, :]
    allow_l = (jj <= ii) & (ii < 128)
    allow_r = (ii >= 128) & (jj > ii - 128)
    mask_std = np.where(allow_l | allow_r, 0.0, MASK_NEG).astype(f)
    mask_j7 = np.where(allow_l, 0.0, MASK_NEG).astype(f)
    maskT = np.concatenate([mask_std, mask_std, mask_std, mask_j7], 1)

    id64 = np.tile(np.eye(64, dtype=f), (2, 1))

    def tile_T(w):  # [E, DIM] -> tiled transposed [128, DT*E]
        E = w.shape[0]
        out = np.zeros((128, DT * E), f)
        for t in range(DT):
            dp = min(128, DIM - 128 * t)
            out[:dp, E * t:E * (t + 1)] = w[:, 128 * t:128 * t + dp].T
        return out

    in_maps = []
    for c in range(NC):
        wq_c = wq_w[EL * c:EL * (c + 1)]                  # [512, 2880]
        wkv_c = np.concatenate([wk_w[HD * c:HD * (c + 1)],
                                wv_w[HD * c:HD * (c + 1)]], 0)  # [128, 2880]
        wo_c = np.ascontiguousarray(wo_w[:, EL * c:EL * (c + 1)].T)  # [512,2880]
        woT_t = np.zeros((128, 4 * DIM), f)
        for et in range(4):
            woT_t[:, DIM * et:DIM * (et + 1)] = wo_c[128 * et:128 * (et + 1)]
        in_maps.append({
            "xT": xT,
            "wqT": tile_T(wq_c).astype(bf),
            "wkvT": tile_T(wkv_c).astype(bf),
            "woT": woT_t.astype(bf),
            "qb": np.ascontiguousarray(
                wq_b[EL * c:EL * (c + 1)].reshape(4, 128).T).astype(f),
            "kvb": np.ascontiguousarray(np.concatenate(
                [wk_b[HD * c:HD * (c + 1)],
                 wv_b[HD * c:HD * (c + 1)]]).reshape(1, 128).T).astype(f),
            "cosq": cosq, "sinq": sinq, "cosk": cosk, "sinkt": sinkt,
            "maskT": maskT,
            "esink": _esink_layout(sinks[HL * c:HL * (c + 1)]),
            "id64": id64,
        })
    return in_maps


def run_on_hw(inputs, trace=False, **kw):
    from concourse import bass_utils
    if "nc" not in _cache:
        _cache["nc"] = _build_module()
    in_maps = _host_prep(**inputs)
    res = bass_utils.run_bass_kernel_spmd(
        _cache["nc"], in_maps, core_ids=list(range(NC)), trace=trace, **kw)
    partials = [res.results[c]["out"].astype(np.float64) for c in range(NC)]
    out = np.sum(np.stack(partials, 0), 0)
    out = (out + inputs["wo_b"].astype(np.float64)).astype(np.float32)
    return out.reshape(B, S, DIM), res


def kernel(**inputs) -> np.ndarray:
    out, _ = run_on_hw(inputs, trace=False)
    return out


# revision 29
# speedup vs baseline: 1.4610x; 1.3452x over previous
"""Sparse (sliding-window + sink) GQA attention block on 8 TRN2 NeuronCores.

Sharding: tensor-parallel over the 64 q-heads -> 8 q-heads (= 1 kv-head
group) per core; x replicated; wo partial outputs summed on host.

v3 dataflow (bf16 storage for DMA-heavy operands, f32 rope/scores path):
  DMA goes through one in-order ring shared by 16 engines and is
  descriptor-rate bound, so every stream uses >=2KB-per-partition-row
  chunks, issued in need-order (wq/x first, tables mid, wo late).
  x^T is DMA'd once ([128,1024] bf16 tiles, resident) and reused by both
  512-col PSUM passes of phase A.  Bias adds on ACT; RoPE on GpSimd
  (overlapped with phase A's second pass).
  Attention per head: scoresT (f32r) -> exp straight off PSUM (ACT,
  bf16) -> multiplicative 0/1 bf16 mask (GpSimd) -> sliding window
  accumulated in PSUM with two 128-col matmuls per i-block; denom row
  via ones-column of v_aug.  Epilogue: denom rows at 32*(h%4) packed in
  two [128,S] tiles; reciprocal_approx_fast (DVE); rinv broadcast by
  ones-matmul; scale into bf16 atb tiles (DVE).
  Phase D: out[i,dd] partial = atb^T wo (bf16), one [128,2880] bf16
  write per i-block; host sums partials.
"""

import numpy as np

B, S, DIM = 1, 1024, 2880
H, HKV, HD = 64, 8, 64
GROUP = H // HKV
WINDOW = 128
THETA = 150000.0
NC = 8
HL = H // NC                 # 8 local q-heads per core
EL = HL * HD                 # 512 local q-dim
DT = (DIM + 127) // 128      # 23 d-tiles (22 full + 64)
NJ = S // 128                # 8 j/i blocks

_cache = {}


def _build_module():
    import concourse.bacc as bacc
    import concourse.mybir as mybir
    import concourse.tile as tile

    f32 = mybir.dt.float32
    f32r = mybir.dt.float32r
    bf16 = mybir.dt.bfloat16
    AF = mybir.ActivationFunctionType
    OP = mybir.AluOpType

    nc = bacc.Bacc("TRN2", target_bir_lowering=False, debug=False)

    def din(name, shape, dt=f32):
        return nc.dram_tensor(name, shape, dt, kind="ExternalInput").ap()

    xT = din("xT", [DIM, S], bf16)             # x^T
    wqT = din("wqT", [128, DT * EL], bf16)     # tiled: [p, 512*t + e]
    wkvT = din("wkvT", [128, DT * 128], bf16)  # tiled: [p, 128*t + (k|v)]
    woT = din("woT", [128, 4 * DIM], bf16)     # tiled: [p, 2880*et + dd]
    qb = din("qb", [128, 4])
    kvb = din("kvb", [128, 1])
    cosq = din("cosq", [128, S])               # 0.125-scaled
    sinq = din("sinq", [128, S])               # 0.125-scaled, sign-baked
    cosk = din("cosk", [64, S])
    sinkt = din("sinkt", [64, S])
    m01 = din("m01", [128, 1024], bf16)        # 0/1 mask [std|std | std|J7]
    esink = din("esink", [128, 2])             # exp(sinks), row 32*(h%4)
    id64 = din("id64", [128, 64])              # eye(64) stacked twice
    out_d = nc.dram_tensor("out", [S, DIM], bf16, kind="ExternalOutput").ap()

    with tile.TileContext(nc) as tc:
        import contextlib
        with contextlib.ExitStack() as ctx:
            res = ctx.enter_context(tc.tile_pool(name="res", bufs=1))
            wq_sb = res.tile([128, DT * EL], bf16, tag="wq")
            wkv_sb = res.tile([128, DT * 128], bf16, tag="wkv")
            wo_sb = res.tile([128, 4 * DIM], bf16, tag="wo")
            xh_all = [res.tile([128, 1024], bf16, tag=f"xh{t}",
                               name=f"xh{t}") for t in range(DT)]
            cq_sb = res.tile([128, S], f32, tag="cq")
            sq_sb = res.tile([128, S], f32, tag="sq")
            ck_sb = res.tile([64, S], f32, tag="ck")
            sk_sb = res.tile([64, S], f32, tag="sk")
            m01_sb = res.tile([128, 1024], bf16, tag="m01")
            qb_sb = res.tile([128, 4], f32, tag="qb")
            kvb_sb = res.tile([128, 1], f32, tag="kvb")
            es_sb = res.tile([128, 2], f32, tag="es")
            id_sb = res.tile([128, 64], f32, tag="id")
            ones0 = res.tile([1, 64], f32, tag="ones0")
            onesr = res.tile([1, 64], f32, tag="onesr")
            z0 = res.tile([128, 128], f32, tag="z0")
            qT = [res.tile([128, S + 128], f32, tag=f"qT{i}", name=f"qT{i}")
                  for i in range(4)]
            kv_sb = res.tile([128, S], f32, tag="kv")
            kv2_sb = res.tile([128, S], f32, tag="kv2")
            v_sb = [res.tile([128, 65], bf16, tag=f"v{j}", name=f"v{j}")
                    for j in range(NJ)]
            at_pair = [res.tile([128, S], f32, tag=f"at{t}", name=f"at{t}")
                       for t in range(4)]
            atb = [res.tile([128, S], bf16, tag=f"ab{t}", name=f"ab{t}")
                   for t in range(4)]
            dn_ab = [res.tile([128, S], f32, tag=f"dn{i}", name=f"dn{i}")
                     for i in range(2)]
            ri_ab = [res.tile([128, S], f32, tag=f"ri{i}", name=f"ri{i}")
                     for i in range(2)]

            nc.vector.memset(ones0[:], 1.0)
            nc.vector.tensor_copy(onesr[:].bitcast(f32r), ones0[:])
            nc.vector.memset(dn_ab[0][:], 1.0)
            nc.vector.memset(dn_ab[1][:], 1.0)
            nc.vector.memset(z0[:], 0.0)
            for et in range(4):
                nc.vector.tensor_copy(qT[et][:, S:S + 128].bitcast(f32r),
                                      z0[:])

            # ---- initial DMAs, in need-order (ring is in-order) ----
            nc.sync.dma_start(wq_sb[:, 0:2 * EL], wqT[:, 0:2 * EL])
            nc.sync.dma_start(wkv_sb[:, 0:128], wkvT[:, 0:128])
            nc.sync.dma_start(xh_all[0][:], xT[0:128, :])
            nc.sync.dma_start(xh_all[1][:], xT[128:256, :])
            nc.sync.dma_start(qb_sb[:], qb[:])
            nc.sync.dma_start(kvb_sb[:], kvb[:])

            # ---------------- Phase A + RoPE, per 512-col half -----------
            with tc.tile_pool(name="pqA", bufs=1, space="PSUM") as pq_pool, \
                 tc.tile_pool(name="pkvA", bufs=1, space="PSUM") as pkv_pool, \
                 tc.tile_pool(name="pvt", bufs=2, space="PSUM") as pvt_pool, \
                 tc.tile_pool(name="rope", bufs=1) as rp:
              for sc in range(2):
                pq = [pq_pool.tile([128, 512], f32, tag=f"pq{e}",
                                   name=f"pq{e}") for e in range(4)]
                pkv = pkv_pool.tile([128, 512], f32, tag="pkv")
                for t in range(DT):
                    dp = 128 if t < DT - 1 else DIM - 128 * (DT - 1)
                    if sc == 0:
                        # paced prefetch through the single DMA ring
                        if t + 2 < DT:
                            dp2 = (128 if t + 2 < DT - 1
                                   else DIM - 128 * (DT - 1))
                            nc.sync.dma_start(
                                xh_all[t + 2][:dp2, :],
                                xT[128 * (t + 2):128 * (t + 2) + dp2, :])
                        if t <= 10:
                            c0 = EL * (2 * t + 2)
                            c1 = min(EL * (2 * t + 4), DT * EL)
                            if c0 < DT * EL:
                                nc.sync.dma_start(wq_sb[:, c0:c1],
                                                  wqT[:, c0:c1])
                        if t in (0, 4, 8):
                            k0 = 128 + 1024 * (t // 4)
                            k1 = min(k0 + 1024, DT * 128)
                            nc.sync.dma_start(wkv_sb[:, k0:k1],
                                              wkvT[:, k0:k1])
                        if t == 12:
                            nc.sync.dma_start(cq_sb[:], cosq[:])
                            nc.sync.dma_start(sq_sb[:], sinq[:])
                        if t == 14:
                            nc.sync.dma_start(ck_sb[:], cosk[:])
                            nc.sync.dma_start(sk_sb[:], sinkt[:])
                            nc.sync.dma_start(m01_sb[:], m01[:])
                            nc.sync.dma_start(es_sb[:], esink[:])
                            nc.sync.dma_start(id_sb[:], id64[:])
                    else:
                        if t in (1, 3, 5, 7, 9, 11):
                            c = t // 2
                            nc.sync.dma_start(
                                wo_sb[:, 1920 * c:1920 * (c + 1)],
                                woT[:, 1920 * c:1920 * (c + 1)])
                    rhs = xh_all[t][:dp, 512 * sc:512 * (sc + 1)]
                    st, sp = (t == 0), (t == DT - 1)
                    for et in range(4):
                        nc.tensor.matmul(
                            pq[et][:],
                            wq_sb[:dp, EL * t + 128 * et:
                                  EL * t + 128 * (et + 1)],
                            rhs, start=st, stop=sp)
                    nc.tensor.matmul(
                        pkv[:], wkv_sb[:dp, 128 * t:128 * (t + 1)],
                        rhs, start=st, stop=sp)
                hs = slice(512 * sc, 512 * (sc + 1))
                for et in range(4):
                    nc.scalar.activation(qT[et][:, hs].bitcast(f32r),
                                         pq[et][:],
                                         AF.Identity, bias=qb_sb[:, et:et + 1])
                nc.scalar.activation(kv_sb[:, hs].bitcast(f32r), pkv[:],
                                     AF.Identity, bias=kvb_sb[:, 0:1])

                # ---- RoPE for this half (GpSimd + swap DMAs), k first
                ksw = rp.tile([64, 512], f32, tag="ksw")
                nc.sync.dma_start(ksw[0:32, :], kv_sb[32:64, hs])
                nc.sync.dma_start(ksw[32:64, :], kv_sb[0:32, hs])
                ktmp = rp.tile([64, 512], f32, tag="ktmp")
                kqc = rp.tile([64, 512], f32, tag="kqc")
                nc.gpsimd.tensor_tensor(ktmp[:], ksw[:], sk_sb[:, hs],
                                        op=OP.mult)
                nc.gpsimd.tensor_tensor(kqc[:], kv_sb[0:64, hs],
                                        ck_sb[:, hs], op=OP.mult)
                nc.gpsimd.tensor_tensor(kv_sb[0:64, hs].bitcast(f32r),
                                        kqc[:], ktmp[:], op=OP.add)
                # kT copy at base 64 for odd heads
                nc.sync.dma_start(kv2_sb[64:128, hs].bitcast(f32r),
                                  kv_sb[0:64, hs].bitcast(f32r))
                # v transposes for this half (PE, tiny) -> bf16 v_sb
                for j in range(4 * sc, 4 * sc + 4):
                    pvt = pvt_pool.tile([128, 64], f32, tag="pvt")
                    nc.tensor.transpose(
                        pvt[:], kv_sb[64:128, 128 * j:128 * (j + 1)],
                        id_sb[64:128, :])
                    nc.vector.tensor_copy(v_sb[j][:, 0:64], pvt[:])
                    nc.vector.memset(v_sb[j][:, 64:65], 1.0)
                # q rope
                for et in range(4):
                    q = qT[et]
                    qsw = rp.tile([128, 512], f32, tag="qsw")
                    nc.sync.dma_start(qsw[0:32, :], q[32:64, hs])
                    nc.sync.dma_start(qsw[32:64, :], q[0:32, hs])
                    nc.sync.dma_start(qsw[64:96, :], q[96:128, hs])
                    nc.sync.dma_start(qsw[96:128, :], q[64:96, hs])
                    tmp = rp.tile([128, 512], f32, tag="tmp")
                    qc = rp.tile([128, 512], f32, tag="qc")
                    nc.gpsimd.tensor_tensor(tmp[:], qsw[:], sq_sb[:, hs],
                                            op=OP.mult)
                    nc.gpsimd.tensor_tensor(qc[:], q[:, hs], cq_sb[:, hs],
                                            op=OP.mult)
                    nc.gpsimd.tensor_tensor(q[:, hs].bitcast(f32r),
                                            qc[:], tmp[:], op=OP.add)

            # ---------------- Phase C: attention ----------------
            with tc.tile_pool(name="pbig", bufs=2, space="PSUM") as pbig_pool, \
                 tc.tile_pool(name="psc", bufs=2, space="PSUM") as ps_pool, \
                 tc.tile_pool(name="prt", bufs=2, space="PSUM") as prt_pool, \
                 tc.tile_pool(name="ee0", bufs=2) as eT0_pool, \
                 tc.tile_pool(name="eet", bufs=3) as eT_pool, \
                 tc.tile_pool(name="stg", bufs=2) as stg_pool:

                def epilogue(g):
                    # rinv for head group g (4 heads), then scale attnT
                    nc.vector.reciprocal_approx_fast(ri_ab[g][:], dn_ab[g][:])
                    for h in range(4 * g, 4 * g + 4):
                        t, r0 = h // 2, 64 * (h % 2)
                        dr = 32 * (h % 4)
                        stg = stg_pool.tile([1, S], f32, tag="stg",
                                            name=f"stg{h}")
                        nc.sync.dma_start(stg[:].bitcast(f32r),
                                          ri_ab[g][dr:dr + 1, :]
                                          .bitcast(f32r))
                        for half in range(2):
                            hs = slice(512 * half, 512 * (half + 1))
                            prt = prt_pool.tile([64, 512], f32, tag="prt")
                            nc.tensor.matmul(
                                prt[:], onesr[0:1, :].bitcast(f32r),
                                stg[0:1, hs].bitcast(f32r),
                                start=True, stop=True)
                            nc.vector.tensor_tensor(
                                atb[t][r0:r0 + 64, hs],
                                at_pair[t][r0:r0 + 64, hs],
                                prt[0:64, :], op=OP.mult)

                for h in range(HL):
                    qt = qT[h // 2]
                    r0 = 64 * (h % 2)
                    tpi = h // 2
                    kt = kv_sb if h % 2 == 0 else kv2_sb
                    pbig = pbig_pool.tile([65, S], f32, tag="pbig")
                    eS = [None] * NJ     # AP slices into pair tiles
                    for Jp in range(NJ // 2):
                        # scores for J-pair (2Jp, 2Jp+1) into one psum tile
                        ps = ps_pool.tile([128, 512], f32, tag="ps")
                        for q2 in range(2):
                            J = 2 * Jp + q2
                            nc.tensor.matmul(
                                ps[:, 256 * q2:256 * (q2 + 1)],
                                kt[r0:r0 + 64, 128 * J:128 * (J + 1)]
                                .bitcast(f32r),
                                qt[r0:r0 + 64, 128 * J:128 * J + 256]
                                .bitcast(f32r),
                                start=True, stop=True)
                        mks = (slice(0, 512) if Jp < NJ // 2 - 1
                               else slice(512, 1024))
                        eT0 = eT0_pool.tile([128, 512], bf16, tag="eT0")
                        nc.scalar.activation(eT0[:], ps[:], AF.Exp)
                        eT = eT_pool.tile([128, 512], bf16, tag="eT")
                        nc.gpsimd.tensor_tensor(eT[:], eT0[:],
                                                m01_sb[:, mks], op=OP.mult)
                        eS[2 * Jp] = eT[:, 0:256]
                        eS[2 * Jp + 1] = eT[:, 256:512]
                        # window-accumulated attnT for i-blocks I=2Jp, 2Jp+1
                        for q2 in range(2):
                            J = 2 * Jp + q2
                            dst = pbig[:, 128 * J:128 * (J + 1)]
                            if J == 0:
                                nc.tensor.matmul(dst, v_sb[0][:, 0:65],
                                                 eS[0][:, 0:128],
                                                 start=True, stop=True)
                            else:
                                nc.tensor.matmul(dst, v_sb[J - 1][:, 0:65],
                                                 eS[J - 1][:, 128:256],
                                                 start=True, stop=False)
                                nc.tensor.matmul(dst, v_sb[J][:, 0:65],
                                                 eS[J][:, 0:128],
                                                 start=False, stop=True)
                    # drain: attnT rows -> at_pair (ACT), denom row -> dn
                    nc.scalar.activation(at_pair[tpi][r0:r0 + 64, :],
                                         pbig[0:64, :], AF.Copy)
                    dr = 32 * (h % 4)
                    nc.scalar.activation(dn_ab[h // 4][dr:dr + 1, :],
                                         pbig[64:65, :], AF.Identity,
                                         bias=es_sb[dr:dr + 1,
                                                    (h // 4):(h // 4) + 1])
                    if h == 3:
                        epilogue(0)
                epilogue(1)

            # ---------------- Phase D: output projection ----------------
            NDD = 6
            DDC = DIM // NDD  # 480
            with tc.tile_pool(name="po", bufs=3, space="PSUM") as po_pool, \
                 tc.tile_pool(name="ob", bufs=2) as ob_pool:
                for it in range(NJ):
                    ob = ob_pool.tile([128, DIM], bf16, tag="ob")
                    for dd in range(NDD):
                        po = po_pool.tile([128, DDC], f32, tag="po")
                        for et in range(4):
                            nc.tensor.matmul(
                                po[:],
                                atb[et][:, 128 * it:128 * (it + 1)],
                                wo_sb[:, DIM * et + DDC * dd:
                                      DIM * et + DDC * (dd + 1)],
                                start=(et == 0), stop=(et == 3))
                        nc.scalar.activation(ob[:, DDC * dd:DDC * (dd + 1)],
                                             po[:], AF.Copy)
                    nc.sync.dma_start(out_d[128 * it:128 * (it + 1), :],
                                      ob[:])

    nc.compile()
    return nc


def _esink_layout(s8):
    out = np.zeros((128, 2), np.float32)
    for h in range(HL):
        out[32 * (h % 4), h // 4] = np.exp(np.float64(s8[h]))
    return out


def _host_prep(x, wq_w, wq_b, wk_w, wk_b, wv_w, wv_b, wo_w, wo_b, sinks):
    """Build per-core input maps (host-side sharding + layout prep)."""
    import ml_dtypes
    f = np.float32
    bf = ml_dtypes.bfloat16
    xT = np.ascontiguousarray(x.reshape(S, DIM).T).astype(bf)      # [2880,1024]

    half = HD // 2
    inv_freq = 1.0 / (THETA ** (np.arange(half, dtype=np.float64) * 2.0 / HD))
    ang = np.arange(S, dtype=np.float64)[:, None] * inv_freq       # [S, 32]
    cos_t = np.cos(ang).T.astype(f)                                # [32, S]
    sin_t = np.sin(ang).T.astype(f)
    cos64 = np.concatenate([cos_t, cos_t], 0)                      # [64, S]
    sin64 = np.concatenate([-sin_t, sin_t], 0)
    scale = np.float32(HD ** -0.5)
    cosq = np.concatenate([cos64, cos64], 0) * scale               # [128, S]
    sinq = np.concatenate([sin64, sin64], 0) * scale
    cosk = cos64
    sinkt = sin64

    jj = np.arange(128)[:, None]
    ii = np.arange(256)[None, :]
    allow_l = (jj <= ii) & (ii < 128)
    allow_r = (ii >= 128) & (jj > ii - 128)
    m_std = (allow_l | allow_r).astype(f)
    m_j7 = allow_l.astype(f)
    m01 = np.concatenate([m_std, m_std, m_std, m_j7], 1).astype(bf)

    id64 = np.tile(np.eye(64, dtype=f), (2, 1))

    def tile_T(w):  # [E, DIM] -> tiled transposed [128, DT*E]
        E = w.shape[0]
        out = np.zeros((128, DT * E), f)
        for t in range(DT):
            dp = min(128, DIM - 128 * t)
            out[:dp, E * t:E * (t + 1)] = w[:, 128 * t:128 * t + dp].T
        return out

    in_maps = []
    for c in range(NC):
        wq_c = wq_w[EL * c:EL * (c + 1)]                  # [512, 2880]
        wkv_c = np.concatenate([wk_w[HD * c:HD * (c + 1)],
                                wv_w[HD * c:HD * (c + 1)]], 0)  # [128, 2880]
        wo_c = np.ascontiguousarray(wo_w[:, EL * c:EL * (c + 1)].T)  # [512,2880]
        woT_t = np.zeros((128, 4 * DIM), f)
        for et in range(4):
            woT_t[:, DIM * et:DIM * (et + 1)] = wo_c[128 * et:128 * (et + 1)]
        in_maps.append({
            "xT": xT,
            "wqT": tile_T(wq_c).astype(bf),
            "wkvT": tile_T(wkv_c).astype(bf),
            "woT": woT_t.astype(bf),
            "qb": np.ascontiguousarray(
                wq_b[EL * c:EL * (c + 1)].reshape(4, 128).T).astype(f),
            "kvb": np.ascontiguousarray(np.concatenate(
                [wk_b[HD * c:HD * (c + 1)],
                 wv_b[HD * c:HD * (c + 1)]]).reshape(1, 128).T).astype(f),
            "cosq": cosq, "sinq": sinq, "cosk": cosk, "sinkt": sinkt,
            "m01": m01,
            "esink": _esink_layout(sinks[HL * c:HL * (c + 1)]),
            "id64": id64,
        })
    return in_maps


def run_on_hw(inputs, trace=False, **kw):
    from concourse import bass_utils
    if "nc" not in _cache:
        _cache["nc"] = _build_module()
    in_maps = _host_prep(**inputs)
    res = bass_utils.run_bass_kernel_spmd(
        _cache["nc"], in_maps, core_ids=list(range(NC)), trace=trace, **kw)
    partials = [res.results[c]["out"].astype(np.float64) for c in range(NC)]
    out = np.sum(np.stack(partials, 0), 0)
    out = (out + inputs["wo_b"].astype(np.float64)).astype(np.float32)
    return out.reshape(B, S, DIM), res


def kernel(**inputs) -> np.ndarray:
    out, _ = run_on_hw(inputs, trace=False)
    return out
